# revision 51
# baseline (speedup 1.0000x reference)
"""Trainium2 Bass kernel for a transformer block (dense_transformer).

Reference computation (B=2, N=2048, C=1024, 16 heads, head_dim=64):
    x = x + attn(LN1(x))            # attn WITHOUT output projection; heads
                                    # interleaved by the faithful reshape
    out = x + MLP(LN2(x))           # MLP = relu(x@w1+b1)@w2+b2

Sharding: 8 cores; core c handles batch b=c//4 and heads 4g..4g+3 (g=c%4).
Because the reference reshapes [B,H,N,hd]->[B,N,C] without transposing
heads back, head h's attention output occupies output rows
[128h, 128h+128) of batch b: out[b, 128h+r, 64m+d] = attn_h[16r+m, d].
So a (batch, 4-head) shard produces a contiguous [512, 1024] output slab
and the whole residual+MLP for those rows is local to the core.

Performance notes (vs the first working version, 609us):
  - The PE HAM clock gate halves the PE clock whenever the engine idles
    >~3.4us, and the old kernel oscillated cold/warm all run.  A warmup
    matmul burst + interleaving QKV matmuls into the LN1 chunk loop +
    prefetching w2 during attention keeps the PE at 2.4 GHz.
  - All four LayerNorm affine params and the v/qkv bias are folded
    host-side: ln1_g into wq/wk/wv rows, ln1_b into the qkv bias,
    ln2_g into w1 rows, ln2_b into b1, and the v-bias directly into the
    residual input xk (softmax rows sum to 1, so attn(v + 1*vb) =
    attn(v) + vb broadcast).  PSUM evacuations become pure copies and
    batch 4..8 transposes per instruction.
  - v is computed directly in natural [seq, d] layout
    (lhsT=ln1T chunk, rhs=wv), killing 64 PE transposes per core.
  - exp on ACT is the attention-phase floor (~121us/core); everything
    else in that phase is kept off ACT.
"""

import os
import sys
from contextlib import ExitStack

for _p in ("/opt/trn_rl_repo", "/root/.axon_site/_ro/trn_rl_repo"):
    if os.path.isdir(_p) and _p not in sys.path:
        sys.path.insert(0, _p)

import numpy as np

import concourse.bass as bass
import concourse.tile as tile
from concourse import bacc, mybir
from concourse.bass_utils import run_bass_kernel_spmd
from concourse.masks import make_identity

F32 = mybir.dt.float32
F32R = mybir.dt.float32r
BF16 = mybir.dt.bfloat16
FP8 = mybir.dt.float8e4
AF = mybir.ActivationFunctionType
OP = mybir.AluOpType

P = 128
B, N, C = 2, 2048, 1024
H, HD = 16, 64
H4 = 4 * C
EPS = 1e-5
SCALE = 1.0 / 32.0  # 1/sqrt(C)

NH = 4            # heads per core
NPAIR = 2         # head pairs per core
ROWS = NH * P     # output rows per core (512)
NCHUNK = N // P   # 16 sequence chunks
CCH = C // P      # 8 channel chunks
HKN = H4 // P     # 32 hidden chunks

_TS = bass.ts


def _emit(nc):
    x = nc.dram_tensor("x", (N, C), F32, kind="ExternalInput").ap()
    xown = nc.dram_tensor("xown", (ROWS, C), F32, kind="ExternalInput").ap()
    wq = nc.dram_tensor("wq", (C, NH * HD), BF16, kind="ExternalInput").ap()
    wk = nc.dram_tensor("wk", (C, NH * HD), BF16, kind="ExternalInput").ap()
    wv = nc.dram_tensor("wv", (C, NH * HD), BF16, kind="ExternalInput").ap()
    qb = nc.dram_tensor("qb", (NH * HD,), F32, kind="ExternalInput").ap()
    kb = nc.dram_tensor("kb", (NH * HD,), F32, kind="ExternalInput").ap()
    w1 = nc.dram_tensor("w1", (C, H4), BF16, kind="ExternalInput").ap()
    b1 = nc.dram_tensor("b1", (H4,), F32, kind="ExternalInput").ap()
    w2 = nc.dram_tensor("w2", (H4, C), BF16, kind="ExternalInput").ap()
    b2 = nc.dram_tensor("b2", (C,), F32R, kind="ExternalInput").ap()
    y = nc.dram_tensor("y", (ROWS, C), F32, kind="ExternalOutput").ap()

    reps = int(os.environ.get("KERNEL_REPS", "1"))
    with tile.TileContext(nc) as tc:
        for _ in range(reps):
            _body(tc, nc, x, xown, wq, wk, wv, qb, kb, w1, b1, w2, b2, y)
    return nc


def _body(tc, nc, x, xown, wq, wk, wv, qb, kb, w1, b1, w2, b2, y):
    with ExitStack() as ctx:
        singles = ctx.enter_context(tc.tile_pool(name="singles", bufs=1))

        # --- constants -------------------------------------------------
        id_b = singles.tile([P, P], BF16)
        make_identity(nc, id_b[:])
        eps_t = singles.tile([P, 1], F32)
        nc.vector.memset(eps_t[:], EPS)
        ones_f = singles.tile([1, P], F32)
        nc.vector.memset(ones_f[:], 1.0)
        ones_row = singles.tile([1, P], F32R)
        nc.vector.tensor_copy(ones_row[:], ones_f[:])

        qb_sb = singles.tile([P, NPAIR], F32)
        nc.sync.dma_start(qb_sb[:], qb.rearrange("(pr p) -> p pr", p=P))
        kb_sb = singles.tile([P, NPAIR], F32)
        nc.sync.dma_start(kb_sb[:], kb.rearrange("(pr p) -> p pr", p=P))
        b1_sb = singles.tile([P, HKN], F32)
        nc.sync.dma_start(b1_sb[:], b1.rearrange("(k p) -> p k", p=P))
        b2_sb = singles.tile([1, C], F32R)
        nc.sync.dma_start(b2_sb[:], b2[None, :])

        # persistent activations spanning attention+MLP
        x2 = singles.tile([P, NH, C], F32)
        ln2T = singles.tile([P, CCH, ROWS], BF16)
        gate = singles.tile([1, 8], BF16)

        # --- HAM warmup: ~18 back-to-back matmuls (~5us) so the PE
        # clock is at 2.4GHz by the time real matmuls start; they run
        # while the first x chunks stream in.
        with (
            tc.tile_pool(name="warm", bufs=1) as wp,
            tc.tile_pool(name="wpp", bufs=1, space="PSUM") as wpp,
        ):
            wsrc = wp.tile([P, 512], BF16)
            nc.vector.memset(wsrc[:], 0.0)
            wps = wpp.tile([P, 512], F32)
            for _ in range(52):
                nc.tensor.matmul(wps[:], id_b[:], wsrc[:],
                                 start=True, stop=True)

        with ExitStack() as actx:
            attn = actx.enter_context(tc.tile_pool(name="attn", bufs=1))
            qT = attn.tile([P, NPAIR, N], BF16)
            # k stored zero-padded per head: head h's 64 d-rows at its
            # native partition offset, the other 64 partitions ZERO.
            # Scores matmuls then present a full 128x128 stationary (the
            # zero rows annihilate the pair-partner's q contribution),
            # which keeps the PE HAM activity monitor at full clock --
            # K=64 matmuls otherwise read as half-utilization and the
            # HAM throttles the whole attention phase to 1.2 GHz.
            kTz = attn.tile([P, NH, N], BF16)
            v_sb = attn.tile([P, NH, NCHUNK, HD + 1], FP8)
            xk = attn.tile([P, NH, C], F32)
            for h in range(NH):
                dz = 0 if h % 2 else HD
                nc.vector.memset(kTz[dz:dz + HD, h, :], 0.0)

            # ------------- phase 1+2: LN1+transpose, QKV --------------
            with (
                tc.tile_pool(name="ph1", bufs=2) as ph1,
                tc.tile_pool(name="pp_a", bufs=4, space="PSUM") as pp_a,
            ):
                ln1T = ph1.tile([P, CCH, N], BF16, tag="ln1T", bufs=1)
                wq_sb = ph1.tile([P, CCH, NH * HD], BF16, tag="wq", bufs=1)
                nc.sync.dma_start(wq_sb[:], wq.rearrange("(k p) m -> p k m", p=P))
                wk_sb = ph1.tile([P, CCH, NH * HD], BF16, tag="wk", bufs=1)
                nc.sync.dma_start(wk_sb[:], wk.rearrange("(k p) m -> p k m", p=P))
                wv_sb = ph1.tile([P, CCH, NH * HD], BF16, tag="wv", bufs=1)
                nc.sync.dma_start(wv_sb[:], wv.rearrange("(k p) m -> p k m", p=P))
                nc.vector.memset(v_sb[:, :, :, HD:HD + 1], 1.0)

                def qk_block(nb):
                    # q/k for seq block [512*nb, 512*nb+512) of all 4 heads
                    for pr in range(NPAIR):
                        for iw, (wsb, bias_sb) in enumerate(
                                ((wq_sb, qb_sb), (wk_sb, kb_sb))):
                            ps = pp_a.tile([P, 512], F32, tag="ps", bufs=3,
                                           name=f"qk{nb}_{pr}_{iw}")
                            for kc in range(CCH):
                                nc.tensor.matmul(
                                    ps[:], wsb[:, kc, _TS(pr, P)],
                                    ln1T[:, kc, _TS(nb, 512)],
                                    start=(kc == 0), stop=(kc == CCH - 1))
                            if iw == 0:
                                nc.scalar.activation(
                                    qT[:, pr, _TS(nb, 512)], ps[:],
                                    AF.Identity,
                                    bias=bias_sb[:, pr:pr + 1], scale=1.0)
                            else:
                                for h2 in range(2):
                                    dp = h2 * HD
                                    nc.scalar.activation(
                                        kTz[dp:dp + HD, 2 * pr + h2,
                                            _TS(nb, 512)],
                                        ps[dp:dp + HD, :], AF.Identity,
                                        bias=bias_sb[dp:dp + HD,
                                                     pr:pr + 1],
                                        scale=1.0)

                for t in range(NCHUNK):
                    x_t = ph1.tile([P, C], F32, tag="xt", bufs=3)
                    nc.sync.dma_start(x_t[:], x[_TS(t, P), :])
                    stats = ph1.tile([P, 2, 6], F32, tag="st")
                    nc.vector.bn_stats(stats[:, 0, :], x_t[:, 0:512])
                    nc.vector.bn_stats(stats[:, 1, :], x_t[:, 512:1024])
                    mv = ph1.tile([P, 2], F32, tag="mv")
                    nc.vector.bn_aggr(mv[:], stats[:])
                    rstd = ph1.tile([P, 1], F32, tag="rs")
                    nc.scalar.activation(rstd[:], mv[:, 1:2], AF.Sqrt,
                                         bias=eps_t[:], scale=1.0)
                    nc.vector.reciprocal(rstd[:], rstd[:])
                    nmr = ph1.tile([P, 1], F32, tag="nm")
                    nc.vector.tensor_scalar(
                        out=nmr[:], in0=mv[:, 0:1], scalar1=rstd[:],
                        scalar2=-1.0, op0=OP.mult, op1=OP.mult)
                    xn = ph1.tile([P, C], BF16, tag="xn")
                    nc.scalar.activation(xn[:], x_t[:], AF.Identity,
                                         bias=nmr[:], scale=rstd[:])
                    # transpose xn -> ln1T, 4 chunks per psum bank, one
                    # pure-copy evacuation per bank (LN affine is folded
                    # into the weights host-side)
                    for half in range(2):
                        pt = pp_a.tile([P, 4, P], BF16, tag="pt", bufs=2,
                                       name=f"pt{t}_{half}")
                        for i in range(4):
                            k = half * 4 + i
                            nc.tensor.transpose(pt[:, i, :], xn[:, _TS(k, P)],
                                                id_b[:])
                        dst = ln1T[:, half * 4:half * 4 + 4, _TS(t, P)]
                        if half == 0:
                            nc.vector.tensor_copy(dst, pt[:])
                        else:
                            nc.scalar.activation(dst, pt[:], AF.Identity,
                                                 scale=1.0)
                    # v for this seq chunk, directly in natural layout:
                    # v[n, d] = (ln1T chunk).T @ wv
                    v_ps = pp_a.tile([P, NH * HD], F32, tag="vps", bufs=2,
                                     name=f"v{t}")
                    for kc in range(CCH):
                        nc.tensor.matmul(
                            v_ps[:], ln1T[:, kc, _TS(t, P)], wv_sb[:, kc, :],
                            start=(kc == 0), stop=(kc == CCH - 1))
                    nc.scalar.activation(
                        v_sb[:, :, t, 0:HD],
                        v_ps.rearrange("p (h d) -> p h d", d=HD),
                        AF.Identity, scale=1.0)
                    if t % 4 == 3:
                        qk_block(t // 4)

            # w2 first-quarter prefetch + xk load run during attention;
            # dispatched from GPSIMD (idle) so they don't queue behind the
            # sync engine's attention-phase semaphore program
            w2r = w2.rearrange("(k p) c -> p k c", p=P)
            w2p = ctx.enter_context(
                tc.tile_pool(name="w2p", bufs=1, side="right"))
            # xk + all of w1 stream during attention.  Each DMA is
            # data-chained behind a gate on qT (phase-1 completion) --
            # a tiny gate-derived write into the destination tile gives
            # the DMA a WAW dependency -- so the transfers can't be
            # scheduled early, where they'd compete with the phase-1
            # x-chunk DMAs.
            nc.gpsimd.tensor_copy(gate[:], qT[0:1, 1, 2040:2048])
            nc.gpsimd.tensor_copy(xk[0:1, 0, 0:8], gate[:])
            nc.gpsimd.dma_start(
                xk[:], xown.rearrange("(h p) c -> p h c", p=P))
            w1r = w1.rearrange("(k p) hh -> p k hh", p=P)
            w1a = w2p.tile([P, CCH, H4 // 2], BF16, name="w1a")
            nc.gpsimd.tensor_copy(w1a[0:1, 0, 0:8], gate[:])
            nc.gpsimd.dma_start(w1a[:], w1r[:, :, 0:H4 // 2])

            # ------------- phase 3: attention per head ----------------
            with (
                tc.tile_pool(name="ph3", bufs=2) as ph3,
                tc.tile_pool(name="pp_s", bufs=2, space="PSUM") as pp_s,
                tc.tile_pool(name="pp_o", bufs=4, space="PSUM") as pp_o,
            ):
                def attn_scores_emitters(h):
                    """expT tiles + 32 emit-callables, each one (mc, nb)
                    scores-matmul pair + its exp evacuation (fp8 out)."""
                    pr, dp = h // 2, (h % 2) * HD
                    expTs = [ph3.tile([P, 8, N], FP8, tag="expT", bufs=6,
                                      name=f"expT{h}_{hf}")
                             for hf in range(2)]

                    def unit(hf, mc8, nb):
                        mc = hf * 8 + mc8
                        pss = pp_s.tile([P, 1024], F32, tag="ss",
                                        name=f"pss{h}_{mc}_{nb}")
                        for nb2 in range(2):
                            nc.tensor.matmul(
                                pss[:, _TS(nb2, 512)],
                                kTz[:, h, _TS(mc, P)],
                                qT[:, pr, _TS(nb * 2 + nb2, 512)],
                                start=True, stop=True)
                        nc.scalar.activation(
                            expTs[hf][:, mc8, _TS(nb, 1024)], pss[:],
                            AF.Exp, scale=SCALE)

                    ems = [lambda a=(hf, mc8, nb): unit(*a)
                           for hf in range(2)
                           for mc8 in range(8)
                           for nb in range(2)]
                    lhss = [t.rearrange("p c (r m) -> p c m r", m=16)
                            for t in expTs]
                    return ems, lhss

                def attn_out(h, lhss, ems=()):
                    # each j owns a contiguous 16-matmul accumulation group:
                    # start=True clears has_written bank-wide, so groups in
                    # a shared bank must not interleave.  The next head's
                    # scores units (ems) are emitted between j-groups so
                    # the PE stays fed while ACT streams exp.
                    pso = [pp_o.tile([P, 4, HD + 1], F32, tag="oo", bufs=4,
                                     name=f"pso{h}_{q}") for q in range(4)]
                    for j in range(16):
                        for mc in range(16):
                            nc.tensor.matmul(
                                pso[j // 4][:, j % 4, :],
                                lhss[mc // 8][:, mc % 8, j, :],
                                v_sb[:, h, mc, :],
                                start=(mc == 0), stop=(mc == 15))
                        for e in ems[2 * j:2 * j + 2]:
                            e()
                    return pso

                def attn_post(h, pso):
                    for j in range(16):
                        rden = ph3.tile([P, 1], F32, tag="rden",
                                        name=f"rden{h}_{j}")
                        nc.vector.reciprocal(
                            rden[:], pso[j // 4][:, j % 4, HD:HD + 1])
                        nc.vector.tensor_scalar(
                            out=x2[:, h, _TS(j, HD)],
                            in0=pso[j // 4][:, j % 4, 0:HD],
                            scalar1=rden[:], scalar2=None, op0=OP.mult)
                    nc.vector.tensor_add(x2[:, h, :], x2[:, h, :],
                                         xk[:, h, :])

                    # LN2 for this block + transpose (pure copy out; LN2
                    # affine folded into w1/b1 host-side)
                    stats2 = ph3.tile([P, 2, 6], F32, tag="st2")
                    nc.vector.bn_stats(stats2[:, 0, :], x2[:, h, 0:512])
                    nc.vector.bn_stats(stats2[:, 1, :], x2[:, h, 512:1024])
                    mv2 = ph3.tile([P, 2], F32, tag="mv2")
                    nc.vector.bn_aggr(mv2[:], stats2[:])
                    rstd2 = ph3.tile([P, 1], F32, tag="rs2")
                    nc.scalar.activation(rstd2[:], mv2[:, 1:2], AF.Sqrt,
                                         bias=eps_t[:], scale=1.0)
                    nc.vector.reciprocal(rstd2[:], rstd2[:])
                    xn2 = ph3.tile([P, C], BF16, tag="xn2")
                    nc.vector.tensor_scalar(
                        out=xn2[:], in0=x2[:, h, :], scalar1=mv2[:, 0:1],
                        scalar2=rstd2[:], op0=OP.subtract, op1=OP.mult)
                    pt2 = pp_s.tile([P, 1024], BF16, tag="ss",
                                    name=f"pt2{h}")
                    pt2v = pt2.rearrange("p (k n) -> p k n", n=P)
                    for k in range(CCH):
                        nc.tensor.transpose(pt2v[:, k, :], xn2[:, _TS(k, P)],
                                            id_b[:])
                    nc.vector.tensor_copy(ln2T[:, :, _TS(h, P)], pt2v[:])

                # software-pipelined: scores/exp of head h+1 are emitted
                # between the attention-output j-groups of head h; the
                # last few units go ahead of attn_post so ACT keeps
                # streaming exp while the PE does the LN2 transposes
                ems, prev = attn_scores_emitters(0)
                for e in ems:
                    e()
                for h in range(1, NH):
                    ems, lh = attn_scores_emitters(h)
                    pso = attn_out(h - 1, prev, ems[:28])
                    for e in ems[28:]:
                        e()
                    attn_post(h - 1, pso)
                    prev = lh
                pso = attn_out(NH - 1, prev)
                attn_post(NH - 1, pso)

        # ---------------- phase 4+5: MLP ------------------------------
        with ExitStack() as mctx:
            mlp = mctx.enter_context(tc.tile_pool(name="mlp", bufs=1))
            h1T = mlp.tile([P, HKN, ROWS], BF16)
            w1b = mlp.tile([P, CCH, H4 // 2], BF16, name="w1b")
            nc.gpsimd.tensor_copy(w1b[0:1, 0, 0:8], gate[:])
            nc.gpsimd.dma_start(w1b[:], w1r[:, :, H4 // 2:H4])
            # w2 quarters rotate through 2 buffers; all data-chained on
            # the phase-1 gate, quarters 2/3 additionally wait (WAR) for
            # the MLP2 pass that frees their slot
            w2qp = mctx.enter_context(tc.tile_pool(name="w2qp", bufs=2))
            w2q = []
            for cq in range(4):
                t_ = w2qp.tile([P, HKN, 256], BF16, tag="w2q",
                               name=f"w2q{cq}")
                nc.gpsimd.tensor_copy(t_[0:1, 0, 0:8], gate[:])
                nc.gpsimd.dma_start(t_[:], w2r[:, :, _TS(cq, 256)])
                w2q.append(t_)
            with tc.tile_pool(name="pp_m", bufs=2, space="PSUM") as pp_m:
                for hk in range(HKN):
                    w1h = w1a if hk < HKN // 2 else w1b
                    ho = hk if hk < HKN // 2 else hk - HKN // 2
                    psh = pp_m.tile([P, ROWS], F32, tag="mm")
                    for kc in range(CCH):
                        nc.tensor.matmul(
                            psh[:], w1h[:, kc, _TS(ho, P)], ln2T[:, kc, :],
                            start=(kc == 0), stop=(kc == CCH - 1))
                    nc.scalar.activation(
                        h1T[:, hk, :], psh[:], AF.Relu,
                        bias=b1_sb[:, hk:hk + 1], scale=1.0)

            with (
                tc.tile_pool(name="ph5", bufs=2) as ph5,
                tc.tile_pool(name="pp_f", bufs=8, space="PSUM") as pp_f,
            ):
                # 8 psum banks hold the full [512, 1024] ff output; each
                # bank is one accumulation group (b2 init + both column
                # quarters), so no start=True after the init matmuls
                psf = [pp_f.tile([P, 512], F32, tag="ff", bufs=8,
                                 name=f"psf{q}") for q in range(8)]
                for q in range(8):
                    nc.tensor.matmul(
                        psf[q][:], ones_row[:], b2_sb[0:1, _TS(q // 4, 512)],
                        start=True, stop=False)
                def evac(cg):
                    for j in range(4):
                        y_sb = ph5.tile([P, 512], F32, tag="ysb")
                        nc.vector.tensor_add(
                            y_sb[:], psf[cg * 4 + j][:],
                            x2[:, j, _TS(cg, 512)])
                        nc.sync.dma_start(y[_TS(j, P), _TS(cg, 512)],
                                          y_sb[:])

                # banks 0-3 (column half 0) finish after quarters 0+1;
                # their evacuation+DMA overlaps the second half's matmuls
                for cq in range(4):
                    for hk in range(HKN):
                        for j in range(4):
                            nc.tensor.matmul(
                                psf[(cq // 2) * 4 + j][:, _TS(cq % 2, 256)],
                                h1T[:, hk, _TS(j, P)],
                                w2q[cq][:, hk, :],
                                start=False,
                                stop=(hk == HKN - 1 and cq % 2 == 1))
                    if cq == 1:
                        evac(0)
                evac(1)


_NC_CACHE = {}


def _get_nc():
    key = os.environ.get("KERNEL_REPS", "1")
    if key not in _NC_CACHE:
        nc = bacc.Bacc("TRN2", target_bir_lowering=False, debug=False,
                       num_devices=8)
        _emit(nc)
        nc.compile()
        _NC_CACHE[key] = nc
    return _NC_CACHE[key]


def make_in_maps(x, qkv_w, qkv_b, w1, b1, w2, b2, ln1_g, ln1_b, ln2_g, ln2_b):
    import ml_dtypes
    x = np.asarray(x, dtype=np.float32)
    qkv_w = np.asarray(qkv_w, dtype=np.float32)
    qkv_b = np.asarray(qkv_b, dtype=np.float32)
    w1 = np.asarray(w1, dtype=np.float32)
    b1 = np.asarray(b1, dtype=np.float32)
    w2 = np.asarray(w2, dtype=np.float32)
    b2 = np.asarray(b2, dtype=np.float32)
    g1 = np.asarray(ln1_g, np.float32)
    bb1 = np.asarray(ln1_b, np.float32)
    g2 = np.asarray(ln2_g, np.float32)
    bb2 = np.asarray(ln2_b, np.float32)

    # Fold LN affine transforms into the downstream weights:
    #   qkv(LN1(x)) = (core1(x) * g1 + bb1) @ W + b
    #               = core1(x) @ (g1[:,None]*W) + (bb1 @ W + b)
    # and likewise LN2 into w1/b1.  The kernel then computes only the
    # core (x-mu)*rstd normalization on-chip.
    qkv_w_eff = g1[:, None] * qkv_w
    qkv_b_eff = qkv_b + bb1 @ qkv_w
    w1_eff = np.ascontiguousarray(
        (g2[:, None] * w1).astype(ml_dtypes.bfloat16))
    b1_eff = b1 + bb2 @ w1
    w2_bf = np.ascontiguousarray(w2.astype(ml_dtypes.bfloat16))

    vb_full = qkv_b_eff[2 * C:]
    in_maps = []
    for core in range(8):
        b, g = divmod(core, 4)
        cs = slice(256 * g, 256 * (g + 1))
        # Fold the v-bias into the residual input: softmax rows sum to 1,
        # so attention(v + 1*vb) = attention(v) + vb broadcast over rows.
        # In the interleaved output layout head h's vb tiles 16x along
        # the channels of its 128-row block.
        xown = x[b, 512 * g:512 * (g + 1)].copy()
        vb_core = vb_full[cs]
        for hl in range(NH):
            pat = np.tile(vb_core[64 * hl:64 * (hl + 1)], 16)
            xown[128 * hl:128 * (hl + 1), :] += pat[None, :]
        in_maps.append({
            "x": np.ascontiguousarray(x[b]),
            "xown": np.ascontiguousarray(xown),
            "wq": np.ascontiguousarray(
                qkv_w_eff[:, cs].astype(ml_dtypes.bfloat16)),
            "wk": np.ascontiguousarray(
                qkv_w_eff[:, C:2 * C][:, cs].astype(ml_dtypes.bfloat16)),
            "wv": np.ascontiguousarray(
                qkv_w_eff[:, 2 * C:][:, cs].astype(ml_dtypes.bfloat16)),
            "qb": np.ascontiguousarray(qkv_b_eff[cs]),
            "kb": np.ascontiguousarray(qkv_b_eff[C:2 * C][cs]),
            "w1": w1_eff, "b1": b1_eff, "w2": w2_bf, "b2": b2,
        })
    return in_maps


def kernel(x, qkv_w, qkv_b, w1, b1, w2, b2, ln1_g, ln1_b, ln2_g, ln2_b):
    nc = _get_nc()
    in_maps = make_in_maps(x, qkv_w, qkv_b, w1, b1, w2, b2,
                           ln1_g, ln1_b, ln2_g, ln2_b)
    res = run_bass_kernel_spmd(nc, in_maps, core_ids=list(range(8)))
    out = np.empty((B, N, C), dtype=np.float32)
    for core in range(8):
        b, g = divmod(core, 4)
        out[b, 512 * g:512 * (g + 1)] = res.results[core]["y"]
    return out
